# revision 7
# baseline (speedup 1.0000x reference)
"""Trainium2 Bass kernel for an activation-gated GCN isotropic layer.

reference:
    hn   = h * norm
    agg  = hn + segment_sum(hn[src], dst, N)
    hh   = agg * norm
    hh   = relu((hh - mean)/sqrt(var + eps) * gamma + beta)   (batch stats)
    return hh, e    (e passes through unchanged)

Distribution (8 NeuronCores, one SPMD program):
  - Nodes padded to N_pad = 8*T*128; core c owns T dst tiles of 128 nodes.
  - Edges sharded by dst tile.  dma_gather (the vectorized SWDGE gather)
    takes int16 indices, so the replicated hn table is addressed as two
    halves (rows < SPLIT and rows >= SPLIT, both < 32768 rows); each dst
    tile's edges are split by src half into K_lo/K_hi chunks of 128 edge
    slots (uniform bounds over all cores/tiles; empty slots: idx=0,
    dstrel=-1).
  - Per tile: two dma_gathers pull the tile's source rows hn[src] into SBUF
    [128 edges, D] chunks; VectorE builds onehot[e,j] = (dstrel[e]==j);
    TensorE accumulates onehot.T @ msgs over the tile's K_lo+K_hi chunks in
    PSUM == segment_sum for those 128 nodes.
  - Epilogue per tile: hh_pre = (hn_own + seg)*norm; per-feature sums of
    hh_pre and hh_pre^2 accumulate into PSUM via ones-vector matmuls.
  - BN stats ([1, 2D] f32) AllReduce across the 8 cores; scale/shift
    broadcast to 128 partitions with a K=1 matmul; a second in-place pass
    applies scale/shift + relu; DMA the shard out.
"""

import math

import numpy as np

import concourse.bacc as bacc
import concourse.bass as bass
import concourse.tile as tile
from concourse import mybir
from concourse import bass_utils
from contextlib import ExitStack

P = 128
NCORES = 8
BN_EPS = 1e-5

F32 = mybir.dt.float32
BF16 = mybir.dt.bfloat16
I32 = mybir.dt.int32
I16 = mybir.dt.int16

AF = mybir.ActivationFunctionType
ALU = mybir.AluOpType


class Cfg:
    def __init__(self, n_nodes, n_feat, n_tiles, k_lo, k_hi, tab_dtype=F32,
                 gather_group=1):
        self.n_nodes = n_nodes          # true node count (for BN mean)
        self.D = n_feat
        self.T = n_tiles                # dst tiles per core
        self.k_lo = k_lo                # chunks/tile with src < SPLIT
        self.k_hi = k_hi                # chunks/tile with src >= SPLIT
        self.tab_dtype = tab_dtype      # dtype of gather table / messages
        self.n_pad = NCORES * n_tiles * P
        self.split = self.n_pad // 2
        assert self.split < 32768 and self.n_pad - self.split < 32768
        self.GG = gather_group          # tiles per dma_gather call

    @property
    def key(self):
        return (self.n_nodes, self.D, self.T, self.k_lo, self.k_hi,
                str(self.tab_dtype), self.GG)


def build_gcn(tc, outs, ins, cfg):
    """Emit the per-core program.  outs/ins are dicts of DRAM APs."""
    nc = tc.nc
    D, T, GG = cfg.D, cfg.T, cfg.GG
    KL, KH = cfg.k_lo, cfg.k_hi
    tdt = cfg.tab_dtype
    assert T % GG == 0

    tab = ins["tab"]            # [n_pad, D]   replicated hn table (tdt)
    out = outs["out"]           # [T*P, D]     f32
    tab_lo = tab[0:cfg.split, :]
    tab_hi = tab[cfg.split:, :]

    with ExitStack() as ctx:
        const = ctx.enter_context(tc.tile_pool(name="const", bufs=1))
        gather_pool = ctx.enter_context(tc.tile_pool(name="gather", bufs=3))
        oh_pool = ctx.enter_context(tc.tile_pool(name="onehot", bufs=8))
        sq_pool = ctx.enter_context(tc.tile_pool(name="sq", bufs=2))
        misc = ctx.enter_context(tc.tile_pool(name="misc", bufs=1))
        psum_acc = ctx.enter_context(
            tc.tile_pool(name="psum_acc", bufs=4, space="PSUM"))
        psum_misc = ctx.enter_context(
            tc.tile_pool(name="psum_misc", bufs=1, space="PSUM"))
        dram = ctx.enter_context(tc.tile_pool(name="dram", bufs=1,
                                              space="DRAM"))

        # ---- constant loads ---------------------------------------------
        iota_t = const.tile([P, P], F32)
        nc.sync.dma_start(out=iota_t[:], in_=ins["iota"])
        # wrapped int16 index streams (see preprocess for layout)
        idxw_lo_t = const.tile([P, T * KL * 8], I16)
        nc.sync.dma_start(out=idxw_lo_t[:], in_=ins["idxw_lo"])
        idxw_hi_t = const.tile([P, T * KH * 8], I16)
        nc.sync.dma_start(out=idxw_hi_t[:], in_=ins["idxw_hi"])
        dstrel_lo_t = const.tile([P, T * KL], F32)
        nc.sync.dma_start(out=dstrel_lo_t[:], in_=ins["dstrel_lo"])
        dstrel_hi_t = const.tile([P, T * KH], F32)
        nc.sync.dma_start(out=dstrel_hi_t[:], in_=ins["dstrel_hi"])
        normt_t = const.tile([P, T], F32)
        nc.sync.dma_start(out=normt_t[:], in_=ins["norm_t"])
        gamma_t = const.tile([1, D], F32)
        nc.sync.dma_start(out=gamma_t[:], in_=ins["gamma"])
        beta_t = const.tile([1, D], F32)
        nc.sync.dma_start(out=beta_t[:], in_=ins["beta"])

        ones_col = const.tile([P, 1], F32)
        nc.vector.memset(ones_col[:], 1.0)
        ones_row = const.tile([1, P], F32)
        nc.vector.memset(ones_row[:], 1.0)

        # hh starts as hn_own, becomes hh_pre, then the final output
        hh = const.tile([P, T * D], F32)
        nc.sync.dma_start(
            out=hh[:].rearrange("p (t d) -> p t d", d=D),
            in_=ins["hn_own"].rearrange("(t p) d -> p t d", p=P),
        )

        stats_s_ps = psum_misc.tile([1, D], F32, space="PSUM")
        stats_q_ps = psum_misc.tile([1, D], F32, space="PSUM")

        # ---- main loop ----------------------------------------------------
        def gather(g_tile, idxw_t, K, src_view, g0):
            n_idx = GG * K * P
            nc.gpsimd.dma_gather(
                out_ap=g_tile[:].rearrange("p (j d) -> p j d", d=D),
                in_ap=src_view,
                idxs_ap=idxw_t[:, g0 * GG * K * 8:(g0 + 1) * GG * K * 8],
                num_idxs=n_idx,
                num_idxs_reg=n_idx,
                elem_size=D,
                # one packet per descriptor: the single-packet coalesced
                # stream would exceed the 64-desc packet ceiling for
                # K*128/16 > 64, which hangs the SDMA engine.  1KB
                # descriptors don't benefit from packet concat anyway.
                single_packet=False,
            )

        g_lo = g_hi = None
        for t in range(T):
            if t % GG == 0:
                g0 = t // GG
                g_lo = gather_pool.tile([P, GG * KL * D], tdt, tag="glo")
                gather(g_lo, idxw_lo_t, KL, tab_lo, g0)
                g_hi = gather_pool.tile([P, GG * KH * D], tdt, tag="ghi")
                gather(g_hi, idxw_hi_t, KH, tab_hi, g0)
            acc = psum_acc.tile([P, D], F32, space="PSUM")
            KT = KL + KH
            for c in range(KT):
                if c < KL:
                    gsrc = g_lo
                    j = (t % GG) * KL + c
                    dcol = dstrel_lo_t[:, t * KL + c: t * KL + c + 1]
                else:
                    gsrc = g_hi
                    j = (t % GG) * KH + (c - KL)
                    ch = c - KL
                    dcol = dstrel_hi_t[:, t * KH + ch: t * KH + ch + 1]
                oh = oh_pool.tile([P, P], tdt, tag="onehot")
                # onehot[e, j] = (iota[j] == dstrel[e]); dstrel=-1 -> zeros
                nc.vector.tensor_scalar(
                    out=oh[:], in0=iota_t[:], scalar1=dcol,
                    scalar2=None, op0=ALU.is_equal,
                )
                nc.tensor.matmul(
                    acc[:], lhsT=oh[:], rhs=gsrc[:, j * D:(j + 1) * D],
                    start=(c == 0), stop=(c == KT - 1),
                )
            # epilogue: hh_pre = (hn_own + seg) * norm
            htile = hh[:, t * D:(t + 1) * D]
            nc.vector.tensor_add(out=htile, in0=htile, in1=acc[:])
            nc.scalar.activation(out=htile, in_=htile, func=AF.Copy,
                                 scale=normt_t[:, t:t + 1])
            # BN stats accumulate
            sq = sq_pool.tile([P, D], F32, tag="sq")
            nc.scalar.activation(out=sq[:], in_=htile, func=AF.Square)
            nc.tensor.matmul(stats_s_ps[:], lhsT=ones_col[:], rhs=htile,
                             start=(t == 0), stop=(t == T - 1))
            nc.tensor.matmul(stats_q_ps[:], lhsT=ones_col[:],
                             rhs=sq[:], start=(t == 0), stop=(t == T - 1))

        # ---- BN stats allreduce -------------------------------------------
        stats_sb = misc.tile([1, 2 * D], F32)
        nc.vector.tensor_copy(out=stats_sb[:, 0:D], in_=stats_s_ps[:])
        nc.vector.tensor_copy(out=stats_sb[:, D:2 * D], in_=stats_q_ps[:])
        cc_in = dram.tile([1, 2 * D], F32)
        cc_out = dram.tile([1, 2 * D], F32, addr_space="Shared")
        nc.gpsimd.dma_start(out=cc_in[:], in_=stats_sb[:])
        nc.gpsimd.collective_compute(
            "AllReduce", ALU.add,
            replica_groups=[list(range(NCORES))],
            ins=[cc_in.opt()], outs=[cc_out.opt()],
        )
        stats_all = misc.tile([1, 2 * D], F32)
        nc.gpsimd.dma_start(out=stats_all[:], in_=cc_out[:])

        # ---- scale/shift row ----------------------------------------------
        inv_n = 1.0 / float(cfg.n_nodes)
        mean_r = misc.tile([1, D], F32)
        nc.scalar.activation(out=mean_r[:], in_=stats_all[:, 0:D],
                             func=AF.Copy, scale=inv_n)
        esq_r = misc.tile([1, D], F32)
        nc.scalar.activation(out=esq_r[:], in_=stats_all[:, D:2 * D],
                             func=AF.Copy, scale=inv_n)
        msq_r = misc.tile([1, D], F32)
        nc.scalar.activation(out=msq_r[:], in_=mean_r[:], func=AF.Square)
        var_r = misc.tile([1, D], F32)
        nc.vector.tensor_tensor(out=var_r[:], in0=esq_r[:], in1=msq_r[:],
                                op=ALU.subtract)
        eps_t = misc.tile([1, 1], F32)
        nc.vector.memset(eps_t[:], BN_EPS)
        std_r = misc.tile([1, D], F32)
        nc.scalar.activation(out=std_r[:], in_=var_r[:], func=AF.Sqrt,
                             bias=eps_t[:])
        rstd_r = misc.tile([1, D], F32)
        nc.vector.reciprocal(out=rstd_r[:], in_=std_r[:])
        ss_row = misc.tile([1, 2 * D], F32)
        nc.vector.tensor_tensor(out=ss_row[:, 0:D], in0=gamma_t[:],
                                in1=rstd_r[:], op=ALU.mult)
        nc.vector.tensor_tensor(out=ss_row[:, D:2 * D], in0=mean_r[:],
                                in1=ss_row[:, 0:D], op=ALU.mult)
        nc.vector.tensor_tensor(out=ss_row[:, D:2 * D], in0=beta_t[:],
                                in1=ss_row[:, D:2 * D], op=ALU.subtract)
        bc_ps = psum_misc.tile([P, 2 * D], F32, space="PSUM")
        nc.tensor.matmul(bc_ps[:], lhsT=ones_row[:], rhs=ss_row[:],
                         start=True, stop=True)
        bc = misc.tile([P, 2 * D], F32)
        nc.vector.tensor_copy(out=bc[:], in_=bc_ps[:])

        # ---- apply BN + relu, write out -----------------------------------
        out3 = out.rearrange("(t p) d -> p t d", p=P)
        for t in range(T):
            htile = hh[:, t * D:(t + 1) * D]
            nc.vector.tensor_tensor(out=htile, in0=htile, in1=bc[:, 0:D],
                                    op=ALU.mult)
            nc.vector.tensor_tensor(out=htile, in0=htile,
                                    in1=bc[:, D:2 * D], op=ALU.add)
            nc.scalar.activation(out=htile, in_=htile, func=AF.Relu)
            nc.sync.dma_start(out=out3[:, t, :], in_=htile)


# -------------------------------------------------------------------------
# host side: program build + input packing
# -------------------------------------------------------------------------

def build_program(cfg):
    nc = bacc.Bacc("TRN2", target_bir_lowering=False, debug=False,
                   num_devices=NCORES)
    D, T = cfg.D, cfg.T
    KL, KH = cfg.k_lo, cfg.k_hi
    ins = {
        "tab": nc.dram_tensor("tab", [cfg.n_pad, D], cfg.tab_dtype,
                              kind="ExternalInput").ap(),
        "hn_own": nc.dram_tensor("hn_own", [T * P, D], F32,
                                 kind="ExternalInput").ap(),
        "idxw_lo": nc.dram_tensor("idxw_lo", [P, T * KL * 8], I16,
                                  kind="ExternalInput").ap(),
        "idxw_hi": nc.dram_tensor("idxw_hi", [P, T * KH * 8], I16,
                                  kind="ExternalInput").ap(),
        "dstrel_lo": nc.dram_tensor("dstrel_lo", [P, T * KL], F32,
                                    kind="ExternalInput").ap(),
        "dstrel_hi": nc.dram_tensor("dstrel_hi", [P, T * KH], F32,
                                    kind="ExternalInput").ap(),
        "norm_t": nc.dram_tensor("norm_t", [P, T], F32,
                                 kind="ExternalInput").ap(),
        "iota": nc.dram_tensor("iota", [P, P], F32,
                               kind="ExternalInput").ap(),
        "gamma": nc.dram_tensor("gamma", [1, D], F32,
                                kind="ExternalInput").ap(),
        "beta": nc.dram_tensor("beta", [1, D], F32,
                               kind="ExternalInput").ap(),
    }
    outs = {
        "out": nc.dram_tensor("out", [T * P, D], F32,
                              kind="ExternalOutput").ap(),
    }
    with tile.TileContext(nc) as tc:
        build_gcn(tc, outs, ins, cfg)
    nc.compile()
    return nc


def _pack_grid(n_cores, T, K, part, col, core, idx_vals, dstrel_vals):
    """Scatter per-edge values into the uniform slot grids."""
    idx_grid = np.zeros((n_cores, P, T * K), np.int16)
    dst_grid = np.full((n_cores, P, T * K), -1.0, np.float32)
    idx_grid[core, part, col] = idx_vals
    dst_grid[core, part, col] = dstrel_vals
    return idx_grid, dst_grid


def _wrap_idx_stream(idx_grid_c, T, K, GG):
    """Build the wrapped/replicated int16 index stream for dma_gather.

    Per gather group gg, the unwrapped stream is slot-order (chunk-major,
    partition-fast): unwrapped[j*128 + p] = idx_grid[p, gg*GG*K + j].
    The ucode reads it wrapped into 16 partitions: w[r, s] = unwrapped
    [s*16 + r], replicated across the eight 16-partition groups.
    """
    blocks = []
    for gg in range(T // GG):
        cols = idx_grid_c[:, gg * GG * K:(gg + 1) * GG * K]   # [P, GG*K]
        unw = cols.T.ravel()                                   # [GG*K*128]
        w = unw.reshape(-1, 16).T                              # [16, S]
        blocks.append(np.tile(w, (8, 1)))                      # [128, S]
    return np.ascontiguousarray(np.concatenate(blocks, axis=1))


def preprocess(h, e, norm, src, dst, gamma, beta, tab_dtype=F32,
               gather_group=1, min_k=None):
    """Shard + pack inputs.  Returns (cfg, in_maps)."""
    h = np.asarray(h, np.float32)
    norm = np.asarray(norm, np.float32).reshape(-1, 1)
    src = np.asarray(src).astype(np.int64)
    dst = np.asarray(dst).astype(np.int64)
    gamma = np.asarray(gamma, np.float32).reshape(1, -1)
    beta = np.asarray(beta, np.float32).reshape(1, -1)

    N, D = h.shape
    n_tiles_total = math.ceil(math.ceil(N / P) / NCORES) * NCORES
    T = n_tiles_total // NCORES
    n_pad = n_tiles_total * P
    split = n_pad // 2

    hn = h * norm  # [N, D] f32

    tile_of_dst = dst // P
    is_hi = src >= split

    # per (tile, half) chunk-count bound
    cnt_lo = np.bincount(tile_of_dst[~is_hi], minlength=n_tiles_total)
    cnt_hi = np.bincount(tile_of_dst[is_hi], minlength=n_tiles_total)
    k_lo = max(1, int(math.ceil(cnt_lo.max() / P)))
    k_hi = max(1, int(math.ceil(cnt_hi.max() / P)))
    if min_k:
        k_lo, k_hi = max(k_lo, min_k), max(k_hi, min_k)
    cfg = Cfg(N, D, T, k_lo, k_hi, tab_dtype=tab_dtype,
              gather_group=gather_group)
    while cfg.T % cfg.GG:
        cfg.GG -= 1

    np_tab = mybir.dt.np(cfg.tab_dtype)
    tab = np.zeros((n_pad, D), np_tab)
    tab[:N] = hn.astype(np_tab)

    hn_pad = np.zeros((n_pad, D), np.float32)
    hn_pad[:N] = hn
    norm_pad = np.zeros((n_pad,), np.float32)
    norm_pad[:N] = norm[:, 0]

    in_maps = [dict() for _ in range(NCORES)]
    for half, sel, K in (("lo", ~is_hi, k_lo), ("hi", is_hi, k_hi)):
        s_h, d_h, t_h = src[sel], dst[sel], tile_of_dst[sel]
        counts = np.bincount(t_h, minlength=n_tiles_total)
        order = np.argsort(t_h, kind="stable")
        starts = np.concatenate([[0], np.cumsum(counts)[:-1]])
        pos = np.arange(len(d_h)) - np.repeat(starts, counts)
        t_sorted = t_h[order]
        chunk = pos // P
        part = pos % P
        core = t_sorted // T
        col = (t_sorted % T) * K + chunk
        idx_vals = (s_h[order] - (split if half == "hi" else 0)).astype(
            np.int16)
        dst_vals = (d_h[order] - t_sorted * P).astype(np.float32)
        idx_grid, dst_grid = _pack_grid(NCORES, T, K, part, col, core,
                                        idx_vals, dst_vals)
        for c in range(NCORES):
            in_maps[c][f"idxw_{half}"] = _wrap_idx_stream(
                idx_grid[c], T, K, cfg.GG)
            in_maps[c][f"dstrel_{half}"] = np.ascontiguousarray(dst_grid[c])

    iota = np.tile(np.arange(P, dtype=np.float32), (P, 1))
    for c in range(NCORES):
        lo, hi = c * T * P, (c + 1) * T * P
        in_maps[c].update({
            "tab": tab,
            "hn_own": hn_pad[lo:hi],
            "norm_t": np.ascontiguousarray(norm_pad[lo:hi].reshape(T, P).T),
            "iota": iota,
            "gamma": gamma,
            "beta": beta,
        })
    return cfg, in_maps


_PROGRAM_CACHE = {}


def _get_program(cfg):
    if cfg.key not in _PROGRAM_CACHE:
        _PROGRAM_CACHE[cfg.key] = build_program(cfg)
    return _PROGRAM_CACHE[cfg.key]


def run(h, e, norm, src, dst, gamma, beta, tab_dtype=F32, trace=False,
        gather_group=1, min_k=None, **run_kwargs):
    cfg, in_maps = preprocess(h, e, norm, src, dst, gamma, beta,
                              tab_dtype=tab_dtype, gather_group=gather_group,
                              min_k=min_k)
    nc = _get_program(cfg)
    res = bass_utils.run_bass_kernel_spmd(
        nc, in_maps, core_ids=list(range(NCORES)), trace=trace, **run_kwargs)
    shards = [res.results[c]["out"] for c in range(NCORES)]
    hh = np.concatenate(shards, axis=0)[:cfg.n_nodes].astype(np.float32)
    return (hh, np.asarray(e)), res


def kernel(h, e, norm, src, dst, gamma, beta):
    out, _ = run(h, e, norm, src, dst, gamma, beta)
    return out
